# revision 16
# baseline (speedup 1.0000x reference)
"""Trainium2 Bass kernel for nn_FC_70205535421369 (gnn_message_passing).

Strategy (8 NeuronCores, SPMD):
  - GCN xw = X @ [W_csi | W_sim] computed node-sharded (1/8 nodes per core).
  - Edge aggregation: host expands edges per output row (self-loops folded as
    pseudo-edges with coef = dinv^2), partitions by src shard, sorts by output
    row, pads into 128-edge chunks.  Device gathers xw[src] rows with indirect
    DMA and scatter-adds via one-hot matmuls into per-branch partial sums
    [4096 x 1024] per core.
  - ReduceScatter(add) per branch combines partials; each core keeps its 512
    batch rows; bias + leaky-relu applied there.
  - Encoder/decoder/output MLP runs batch-sharded in feature-major layout
    (weights are the matmul stationary operand).  BatchNorm (training mode)
    stats via per-feature sum/sumsq + small AllReduce; the linear bias before
    each BN folds away exactly.
  - Host does sharding, transposes, edge prep and final assembly only.
"""

import os
import sys
import types

import numpy as np

# ---------------------------------------------------------------------------
# Problem constants (hardcoded per spec)
# ---------------------------------------------------------------------------
B = 4096
ND, NP = 10000, 20000
ED, EP = 320000, 640000
PG = 2048
DIM = 300 + 1024 + 2048 + 2048  # 5420
EPS = 1e-5

N_CORES = 8
BS = B // N_CORES  # 512 batch rows per core
P = 128

NDs = 1280  # padded d-node shard size (10240 total)
NPs = 2560  # padded p-node shard size (20480 total)
F = 1024  # GCN output features per branch
NBR = 4  # branches: 0=d_csi(i_ecfps) 1=p_csi(i_gos) 2=d_sim(s_ecfps) 3=p_sim(s_gos)
GCNF = NBR * F  # 4096

# padded feature layout: [0:300 dv][300:384 pad][384:1408 pe][1408:5504 gcn]
DVP = 384
DIMP = DVP + 1024 + GCNF  # 5504
assert DIMP % P == 0

WIN = P  # aggregation row-window size
NWIN = B // WIN  # 32 windows

# dtype knobs (numpy side).  bf16 via ml_dtypes where enabled.
import ml_dtypes

BF16 = ml_dtypes.bfloat16
AGG_BF16 = os.environ.get("K_AGG_BF16", "1") == "1"  # xw store + gather + S
PART_BF16 = os.environ.get("K_PART_BF16", "1") == "1"  # partial sums + RS
MLP_BF16 = os.environ.get("K_MLP_BF16", "0") == "1"  # MLP weights + acts


def _cast(x, bf16):
    if bf16:
        return x.astype(BF16).astype(np.float32)
    return x.astype(np.float32)


# ---------------------------------------------------------------------------
# Host-side preprocessing
# ---------------------------------------------------------------------------
def _expand_edges(idx, ei, ew, n_nodes, ns_pad):
    """Expand graph edges into per-output-row contribution lists.

    Returns per-core chunked arrays:
      esrc  [nch, 128] int32 : src node LOCAL row within the core's xw shard
      erow  [nch, 128] f32   : output row offset within its 128-row window
      ecoef [nch, 128] f32   : contribution coefficient
      win_chunks: list[32]   : chunks per window
    """
    idx = np.asarray(idx, dtype=np.int64)
    src = np.asarray(ei[0], dtype=np.int64)
    dst = np.asarray(ei[1], dtype=np.int64)
    ew = np.asarray(ew, dtype=np.float32)

    deg = np.zeros(n_nodes, np.float32)
    np.add.at(deg, dst, ew)
    deg += 1.0
    dinv = (1.0 / np.sqrt(deg)).astype(np.float32)
    coef = dinv[src] * ew * dinv[dst]

    # rows grouped by node id
    order = np.argsort(idx, kind="stable")
    sidx = idx[order]
    left = np.searchsorted(sidx, dst)
    right = np.searchsorted(sidx, dst, side="right")
    cnt = (right - left).astype(np.int64)
    tot = int(cnt.sum())
    esrc = np.repeat(src, cnt)
    ecoef = np.repeat(coef, cnt)
    starts = np.zeros_like(cnt)
    np.cumsum(cnt[:-1], out=starts[1:])
    offs = np.arange(tot, dtype=np.int64) - np.repeat(starts, cnt) + np.repeat(left, cnt)
    ej = order[offs]

    # self loops
    esrc = np.concatenate([esrc, idx])
    ej = np.concatenate([ej, np.arange(B, dtype=np.int64)])
    ecoef = np.concatenate([ecoef, (dinv[idx] * dinv[idx]).astype(np.float32)])

    # split per core (by src shard), group per window; pad chunk counts to the
    # max across cores so the SPMD program structure is identical on all cores.
    core_of = esrc // ns_pad
    raw = []
    for c in range(N_CORES):
        m = core_of == c
        cs, cj, cc = esrc[m] % ns_pad, ej[m], ecoef[m]
        o = np.argsort(cj, kind="stable")
        cs, cj, cc = cs[o], cj[o], cc[o]
        w = cj // WIN
        wins = []
        for wi in range(NWIN):
            wm = w == wi
            wins.append((cs[wm], cj[wm] - wi * WIN, cc[wm]))
        raw.append(wins)

    win_chunks = [
        max((len(raw[c][wi][0]) + P - 1) // P for c in range(N_CORES))
        for wi in range(NWIN)
    ]
    per_core = []
    for c in range(N_CORES):
        chunks_s, chunks_r, chunks_c = [], [], []
        for wi in range(NWIN):
            cs, cr, cc = raw[c][wi]
            n = len(cs)
            npad = win_chunks[wi] * P - n
            chunks_s.append(np.concatenate([cs, np.zeros(npad, np.int64)]).reshape(-1, P))
            chunks_r.append(np.concatenate([cr, np.zeros(npad, np.int64)]).reshape(-1, P))
            chunks_c.append(np.concatenate([cc, np.zeros(npad, np.float32)]).reshape(-1, P))
        # transpose to [P, nch_total] so the device DMA is contiguous
        per_core.append(
            dict(
                esrc=np.ascontiguousarray(np.concatenate(chunks_s).astype(np.int32).T),
                erow=np.ascontiguousarray(np.concatenate(chunks_r).astype(np.float32).T),
                ecoef=np.ascontiguousarray(np.concatenate(chunks_c).astype(np.float32).T),
                win_chunks=win_chunks,
            )
        )
    return per_core


def _pad_rows(a, n):
    out = np.zeros((n,) + a.shape[1:], a.dtype)
    out[: a.shape[0]] = a
    return out


def _feat_pad_rows(w):
    """Map [5420, ...] -> [5504, ...] padded feature layout."""
    out = np.zeros((DIMP,) + w.shape[1:], np.float32)
    out[0:300] = w[0:300]
    out[DVP : DVP + 1024] = w[300:1324]
    out[DVP + 1024 :] = w[1324:5420]
    return out


def _feat_unpad_cols(a):
    """Inverse along axis 1: [., 5504] -> [., 5420]."""
    return np.concatenate([a[:, 0:300], a[:, DVP:]], axis=1)


def prep_inputs(
    d_index, p_index, d_vecs, p_embeddings, d_ecfps, p_gos,
    d_inter_ei, d_inter_ew, d_sim_ei, d_sim_ew,
    p_inter_ei, p_inter_ew, p_sim_ei, p_sim_ew, params,
):
    """Shard + preprocess all inputs.  Returns (per_core_inputs, meta)."""
    Pm = {k: np.asarray(v, np.float32) for k, v in params.items()}
    d_vecs = np.asarray(d_vecs, np.float32)
    p_embeddings = np.asarray(p_embeddings, np.float32)
    d_ecfps = np.asarray(d_ecfps, np.float32)
    p_gos = np.asarray(p_gos, np.float32)

    # node-feature shards, transposed: [K, ns_pad]
    xd = _pad_rows(d_ecfps, NDs * N_CORES)  # [10240, 1024]
    xp = _pad_rows(p_gos, NPs * N_CORES)  # [20480, 2048]

    # branch configs: (branch, graph kind, idx, ei, ew)
    edges = [
        _expand_edges(d_index, d_inter_ei, d_inter_ew, ND, NDs),  # br0 i_ecfps
        _expand_edges(p_index, p_inter_ei, p_inter_ew, NP, NPs),  # br1 i_gos
        _expand_edges(d_index, d_sim_ei, d_sim_ew, ND, NDs),  # br2 s_ecfps
        _expand_edges(p_index, p_sim_ei, p_sim_ew, NP, NPs),  # br3 s_gos
    ]

    w_d = np.concatenate([Pm["ecfps_csi_W"], Pm["ecfps_sim_W"]], axis=1)  # [1024, 2048]
    w_p = np.concatenate([Pm["gos_csi_W"], Pm["gos_sim_W"]], axis=1)  # [2048, 2048]
    gcn_bias = np.concatenate(
        [Pm["ecfps_csi_b"], Pm["gos_csi_b"], Pm["ecfps_sim_b"], Pm["gos_sim_b"]]
    ).reshape(1, GCNF)

    w1 = _feat_pad_rows(Pm["enc1_W"])  # [5504, 2048]
    w4 = _feat_pad_rows(Pm["dec2_W"].T).T.copy()  # [2048, 5504]
    bn_g4 = _feat_pad_rows(Pm["dec2_g"])
    bn_g4[300:DVP] = 1.0
    bn_b4 = _feat_pad_rows(Pm["dec2_be"])

    def dev_vec(v):
        # [dout] -> [P, dout//P] partition-major device layout
        return np.ascontiguousarray(np.asarray(v, np.float32).reshape(-1, P).T)

    mlp_common = dict(
        w_d=w_d, w_p=w_p, gcn_bias=gcn_bias,
        w1=w1, w2=Pm["enc2_W"], w3=Pm["dec1_W"], w4=w4,
        w5=Pm["out1_W"], w6=Pm["out2_W"],
        bn_g1=dev_vec(Pm["enc1_g"]), bn_b1=dev_vec(Pm["enc1_be"]),
        bn_g2=dev_vec(Pm["enc2_g"]), bn_b2=dev_vec(Pm["enc2_be"]),
        bn_g3=dev_vec(Pm["dec1_g"]), bn_b3=dev_vec(Pm["dec1_be"]),
        bn_g4=dev_vec(bn_g4), bn_b4=dev_vec(bn_b4),
        bn_g5=dev_vec(Pm["out1_g"]), bn_b5=dev_vec(Pm["out1_be"]),
    )
    b6 = float(np.asarray(Pm["out2_b"]).reshape(-1)[0])

    per_core = []
    for c in range(N_CORES):
        ci = dict(mlp_common)
        ci["xt_d"] = np.ascontiguousarray(xd[c * NDs : (c + 1) * NDs].T)  # [1024,1280]
        ci["xt_p"] = np.ascontiguousarray(xp[c * NPs : (c + 1) * NPs].T)  # [2048,2560]
        dv = d_vecs[c * BS : (c + 1) * BS]  # [512, 300]
        ci["dv_t"] = _pad_rows(np.ascontiguousarray(dv.T), DVP)  # [384, 512]
        ci["pe_t"] = np.ascontiguousarray(p_embeddings[c * BS : (c + 1) * BS].T)
        for br in range(NBR):
            e = edges[br][c]
            ci[f"esrc_{br}"] = e["esrc"]
            ci[f"erow_{br}"] = e["erow"]
            ci[f"ecoef_{br}"] = e["ecoef"]
        ci["iota_t"] = np.broadcast_to(
            np.arange(P, dtype=np.float32)[None, :], (P, P)
        ).copy()
        ci["ones_t"] = np.ones((1, P), np.float32)
        per_core.append(ci)

    meta = dict(
        win_chunks=[edges[br][0]["win_chunks"] for br in range(NBR)],
        b6=b6,
    )
    return per_core, meta


# ---------------------------------------------------------------------------
# Numpy emulation of the device program (for validation)
# ---------------------------------------------------------------------------
def _emulate(per_core, meta):
    agg = lambda x: _cast(x, AGG_BF16)
    part = lambda x: _cast(x, PART_BF16)
    mlp = lambda x: _cast(x, MLP_BF16)

    partials = [np.zeros((B, F), np.float32) for _ in range(NBR)]
    xw_store = []
    for c in range(N_CORES):
        ci = per_core[c]
        xw_d = agg(ci["xt_d"].T @ ci["w_d"])  # [1280, 2048]
        xw_p = agg(ci["xt_p"].T @ ci["w_p"])  # [2560, 2048]
        xw_store.append(
            [xw_d[:, :F], xw_p[:, :F], xw_d[:, F:], xw_p[:, F:]]
        )

    for br in range(NBR):
        core_parts = []
        for c in range(N_CORES):
            ci = per_core[c]
            xw = xw_store[c][br]
            pb = np.zeros((B, F), np.float32)
            esrc, erow, ecoef = ci[f"esrc_{br}"], ci[f"erow_{br}"], ci[f"ecoef_{br}"]
            ch = 0
            for w in range(NWIN):
                for _ in range(meta["win_chunks"][br][w]):
                    g = xw[esrc[:, ch]]  # [128, F] (AGG dtype values)
                    s = np.zeros((P, P), np.float32)
                    s[np.arange(P), erow[:, ch].astype(np.int64)] = ecoef[:, ch]
                    s = agg(s)
                    pb[w * WIN : (w + 1) * WIN] += s.T @ g
                    ch += 1
            core_parts.append(part(pb))
        # emulate bf16 reduce-scatter (sequential adds in PART dtype)
        tot = core_parts[0]
        for c in range(1, N_CORES):
            tot = part(tot + core_parts[c])
        partials[br] = tot

    outs = []
    for c in range(N_CORES):
        ci = per_core[c]
        rows = slice(c * BS, (c + 1) * BS)
        gcn = np.concatenate([partials[br][rows] for br in range(NBR)], axis=1)
        gcn = gcn + ci["gcn_bias"]
        gcn = np.where(gcn > 0, gcn, 0.01 * gcn).astype(np.float32)  # [512, 4096]

        featT = np.concatenate([ci["dv_t"], ci["pe_t"], gcn.T], axis=0)  # [5504,512]

        stats_bufs = {}

        def dense_bn(h, w, g, be, act, lname):
            z = mlp(h).T @ mlp(w)  # [512, dout]
            stats_bufs[lname] = np.stack([z.sum(0), (z * z).sum(0)])
            return z.T  # feature-major pre-act; normalized later

        outs.append(dict(ci=ci, featT=featT, stats=stats_bufs))

    # layer-by-layer with cross-core stats (mimics AllReduce)
    def layer(prev_key, w_key, g_key, b_key, act, out_key):
        zs = []
        for o in outs:
            h = o[prev_key]
            z = (mlp(h).T @ mlp(o["ci"][w_key])).T  # [dout, 512]
            zs.append(z)
        S = sum(z.sum(1) for z in zs)
        SQ = sum((z * z).sum(1) for z in zs)
        m = S / B
        v = SQ / B - m * m
        inv = 1.0 / np.sqrt(v + EPS)
        for o, z in zip(outs, zs):
            g = o["ci"][g_key].T.reshape(-1)
            be = o["ci"][b_key].T.reshape(-1)
            s = (g * inv).astype(np.float32)
            t = (be - m * s).astype(np.float32)
            zn = z * s[:, None] + t[:, None]
            if act == "relu":
                zn = np.maximum(zn, 0)
            else:
                zn = np.where(zn > 0, zn, 0.01 * zn)
            o[out_key] = zn.astype(np.float32)

    layer("featT", "w1", "bn_g1", "bn_b1", "relu", "h1")
    layer("h1", "w2", "bn_g2", "bn_b2", "relu", "enc")
    layer("enc", "w3", "bn_g3", "bn_b3", "relu", "h3")
    layer("h3", "w4", "bn_g4", "bn_b4", "relu", "dec")
    layer("enc", "w5", "bn_g5", "bn_b5", "lrelu", "h5")

    results = []
    for c, o in enumerate(outs):
        y = (mlp(o["h5"]).T @ mlp(o["ci"]["w6"])).T + meta["b6"]  # [1, 512]
        rows = slice(c * BS, (c + 1) * BS)
        gcn = np.concatenate([partials[br][rows] for br in range(NBR)], axis=1)
        gcn = gcn + o["ci"]["gcn_bias"]
        gcn = np.where(gcn > 0, gcn, 0.01 * gcn).astype(np.float32)
        results.append(
            dict(
                gcn_feat=gcn.astype(np.float32),
                enc_t=o["enc"],
                dec_t=o["dec"],
                y_t=y.astype(np.float32),
            )
        )
    return results


# ---------------------------------------------------------------------------
# Assembly of final outputs from per-core device results
# ---------------------------------------------------------------------------
def _assemble(results, d_vecs, p_embeddings):
    feature = np.concatenate(
        [
            np.asarray(d_vecs, np.float32),
            np.asarray(p_embeddings, np.float32),
            np.concatenate([r["gcn_feat"] for r in results], axis=0),
        ],
        axis=1,
    )
    encoded = np.concatenate([r["enc_t"].T for r in results], axis=0)
    decoded = np.concatenate(
        [_feat_unpad_cols(r["dec_t"].T) for r in results], axis=0
    )
    y = np.concatenate([r["y_t"].T for r in results], axis=0)
    return (
        y.astype(np.float32),
        encoded.astype(np.float32),
        decoded.astype(np.float32),
        feature.astype(np.float32),
    )


# ---------------------------------------------------------------------------
# Public entry point
# ---------------------------------------------------------------------------
def kernel(**inputs):
    per_core, meta = prep_inputs(**inputs)
    if os.environ.get("K_EMULATE", "0") == "1":
        results = _emulate(per_core, meta)
    else:
        results, _ = _run_on_hw(per_core, meta, trace=False)
    return _assemble(results, inputs["d_vecs"], inputs["p_embeddings"])


def kernel_traced(**inputs):
    """Like kernel() but runs with NTFF tracing; returns (outputs, exec_ns)."""
    per_core, meta = prep_inputs(**inputs)
    results, exec_ns = _run_on_hw(per_core, meta, trace=True)
    return _assemble(results, inputs["d_vecs"], inputs["p_embeddings"]), exec_ns


# ---------------------------------------------------------------------------
# Bass device program
# ---------------------------------------------------------------------------
_HOOK_DONE = False


def _install_ntff_hook():
    """Register the axon NTFF profiling hook (the container's antenv stub
    lacks antenv.axon_hooks, so bass_utils can't find it on its own)."""
    global _HOOK_DONE
    if _HOOK_DONE:
        return
    _HOOK_DONE = True
    try:
        import antenv
        import trn_agent_boot.trn_boot as tb

        hook = tb._ntff_profile_via_ctypes("/opt/axon/libaxon_pjrt.so")
        mod = types.ModuleType("antenv.axon_hooks")
        mod.get_axon_ntff_profile_hook = lambda: hook
        mod.set_axon_ntff_profile_hook = lambda h: None
        antenv.axon_hooks = mod
        sys.modules["antenv.axon_hooks"] = mod
    except Exception:
        pass


_PROG_CACHE = {}

GRAPHS = dict(
    d=dict(K=1024, NS=NDs, KC=1024 // P, NT=NDs // P),
    p=dict(K=2048, NS=NPs, KC=2048 // P, NT=NPs // P),
)
BR_GRAPH = ["d", "p", "d", "p"]  # branch -> graph kind
BR_COL = [0, 0, 1, 1]  # branch -> csi(0)/sim(1) column half of xw


def _build_program(meta):
    import concourse.bass as bass
    import concourse.mybir as mybir
    import concourse.tile as tile
    from concourse import bacc
    from concourse.masks import make_identity

    f32 = mybir.dt.float32
    i32 = mybir.dt.int32
    DT_AGG = mybir.dt.bfloat16 if AGG_BF16 else f32
    DT_PART = mybir.dt.bfloat16 if PART_BF16 else f32
    AF = mybir.ActivationFunctionType
    RG = [list(range(N_CORES))]

    win_chunks = meta["win_chunks"]  # [br][win]
    nch_tot = [sum(win_chunks[br]) for br in range(NBR)]

    nc = bacc.Bacc("TRN2", target_bir_lowering=False, debug=False, num_devices=N_CORES)

    # ---- I/O -------------------------------------------------------------
    def inp(name, shape, dt=f32):
        return nc.dram_tensor(name, shape, dt, kind="ExternalInput")

    xt = {"d": inp("xt_d", [1024, NDs]), "p": inp("xt_p", [2048, NPs])}
    w_g = {"d": inp("w_d", [1024, 2 * F]), "p": inp("w_p", [2048, 2 * F])}
    gcn_bias = inp("gcn_bias", [1, GCNF])
    esrc = [inp(f"esrc_{br}", [P, nch_tot[br]], i32) for br in range(NBR)]
    erow = [inp(f"erow_{br}", [P, nch_tot[br]]) for br in range(NBR)]
    ecoef = [inp(f"ecoef_{br}", [P, nch_tot[br]]) for br in range(NBR)]
    dv_t = inp("dv_t", [DVP, BS])
    pe_t = inp("pe_t", [1024, BS])
    w1 = inp("w1", [DIMP, 2048])
    w2 = inp("w2", [2048, 1024])
    w3 = inp("w3", [1024, 2048])
    w4 = inp("w4", [2048, DIMP])
    w5 = inp("w5", [1024, 256])
    w6 = inp("w6", [256, 1])
    bn_g = {}
    bn_b = {}
    for i, dout in [(1, 2048), (2, 1024), (3, 2048), (4, DIMP), (5, 256)]:
        bn_g[i] = inp(f"bn_g{i}", [dout])
        bn_b[i] = inp(f"bn_b{i}", [dout])
    iota_in = inp("iota_t", [P, P])
    ones_in = inp("ones_t", [1, P])

    gcn_feat = nc.dram_tensor("gcn_feat", [BS, GCNF], f32, kind="ExternalOutput")
    enc_t = nc.dram_tensor("enc_t", [1024, BS], f32, kind="ExternalOutput")
    dec_t = nc.dram_tensor("dec_t", [DIMP, BS], f32, kind="ExternalOutput")
    y_t = nc.dram_tensor("y_t", [1, BS], f32, kind="ExternalOutput")

    # ---- internal DRAM ---------------------------------------------------
    # separate tensors per branch: indirect-DMA gather requires AP offset 0.
    xw = [
        nc.dram_tensor(f"xw_{br}", [GRAPHS[BR_GRAPH[br]]["NS"], F], DT_AGG, kind="Internal")
        for br in range(NBR)
    ]
    partial = [
        nc.dram_tensor(f"partial_{br}", [B, F], DT_PART, kind="Internal")
        for br in range(NBR)
    ]
    recv = [
        nc.dram_tensor(f"recv_{br}", [BS, F], DT_PART, kind="Internal")
        for br in range(NBR)
    ]
    stats_in = {}
    stats_out = {}
    for i, dout in [(1, 2048), (2, 1024), (3, 2048), (4, DIMP), (5, 256)]:
        ot = dout // P
        stats_in[i] = nc.dram_tensor(f"stats_in{i}", [2 * P, ot], f32, kind="Internal")
        stats_out[i] = nc.dram_tensor(
            f"stats_out{i}", [2 * P, ot], f32, kind="Internal", addr_space="Shared"
        )
    featT = nc.dram_tensor("featT", [DIMP, BS], f32, kind="Internal")

    with tile.TileContext(nc) as tc, tc.tile_pool(name="const", bufs=1) as cp:
        # ================= consts =================
        identity = cp.tile([P, P], f32)
        make_identity(nc, identity[:])
        iota_sb = cp.tile([P, P], f32)
        nc.sync.dma_start(iota_sb[:], iota_in[:])
        ones_sb = cp.tile([1, P], f32)
        nc.sync.dma_start(ones_sb[:], ones_in[:])
        eps_sb = cp.tile([P, 1], f32)
        nc.gpsimd.memset(eps_sb[:], float(EPS))

        # ================= phase A: xw = X @ [Wc|Ws] =================
        for gk in ["d", "p"]:
            G = GRAPHS[gk]
            KC, NT = G["KC"], G["NT"]
            xw_c = xw[0] if gk == "d" else xw[1]
            xw_s = xw[2] if gk == "d" else xw[3]
            with (
                tc.tile_pool(name=f"wg_{gk}", bufs=1) as wp,
                tc.tile_pool(name=f"xt_{gk}", bufs=4) as xp,
                tc.tile_pool(name=f"xwst_{gk}", bufs=3) as sp,
                tc.tile_pool(name=f"xwps_{gk}", bufs=1, space="PSUM") as pp,
            ):
                w_sb = wp.tile([P, KC, 2 * F], f32)
                for kc in range(KC):
                    nc.sync.dma_start(w_sb[:, kc, :], w_g[gk][kc * P : (kc + 1) * P, :])
                for nt0 in range(0, NT, 2):
                    nts = [nt0, nt0 + 1]
                    xts = []
                    psums = []
                    for nt in nts:
                        xtt = xp.tile([P, KC, P], f32, tag="xt")
                        for kc in range(KC):
                            nc.sync.dma_start(
                                xtt[:, kc, :],
                                xt[gk][kc * P : (kc + 1) * P, nt * P : (nt + 1) * P],
                            )
                        xts.append(xtt)
                        j2_ = len(xts) - 1
                        psums.append(
                            [
                                pp.tile(
                                    [P, 512], f32,
                                    tag=f"ps{j2_}_{j}", name=f"xwps{j2_}_{j}",
                                )
                                for j in range(4)
                            ]
                        )
                    for kc in range(KC):
                        for j2, nt in enumerate(nts):
                            for ns in range(4):
                                nc.tensor.matmul(
                                    psums[j2][ns][:],
                                    lhsT=xts[j2][:, kc, :],
                                    rhs=w_sb[:, kc, ns * 512 : (ns + 1) * 512],
                                    start=(kc == 0),
                                    stop=(kc == KC - 1),
                                )
                    for j2, nt in enumerate(nts):
                        stage = sp.tile([P, 2 * F], DT_AGG, tag="stage")
                        for ns in range(4):
                            nc.vector.tensor_copy(
                                stage[:, ns * 512 : (ns + 1) * 512], psums[j2][ns][:]
                            )
                        nc.sync.dma_start(
                            xw_c[nt * P : (nt + 1) * P, :], stage[:, 0:F]
                        )
                        nc.sync.dma_start(
                            xw_s[nt * P : (nt + 1) * P, :], stage[:, F : 2 * F]
                        )

        # ================= phase B+C: aggregation + reduce-scatter =========
        with (
            tc.tile_pool(name="meta", bufs=1) as mp,
            tc.tile_pool(name="gat", bufs=6) as gp,
            tc.tile_pool(name="sel", bufs=6) as selp,
            tc.tile_pool(name="aggst", bufs=3) as asp,
            tc.tile_pool(name="aggps", bufs=3, space="PSUM") as app,
        ):
            for br in [0, 2, 1, 3]:  # d branches first (their xw finishes first)
                nch = nch_tot[br]
                esrc_sb = mp.tile([P, nch], i32, tag=f"esrc{br}")
                erow_sb = mp.tile([P, nch], f32, tag=f"erow{br}")
                ecoef_sb = mp.tile([P, nch], f32, tag=f"ecoef{br}")
                nc.sync.dma_start(esrc_sb[:], esrc[br][:])
                nc.sync.dma_start(erow_sb[:], erow[br][:])
                nc.sync.dma_start(ecoef_sb[:], ecoef[br][:])
                ch = 0
                for w in range(NWIN):
                    n = win_chunks[br][w]
                    psum = app.tile([P, F], f32, tag="aggps")
                    for i in range(n):
                        g_tile = gp.tile([P, F], DT_AGG, tag="g")
                        nc.gpsimd.indirect_dma_start(
                            out=g_tile[:],
                            out_offset=None,
                            in_=xw[br][:],
                            in_offset=bass.IndirectOffsetOnAxis(
                                ap=esrc_sb[:, ch : ch + 1], axis=0
                            ),
                        )
                        sel = selp.tile([P, P], f32, tag="sel")
                        nc.vector.tensor_tensor(
                            out=sel[:],
                            in0=erow_sb[:, ch : ch + 1].to_broadcast([P, P]),
                            in1=iota_sb[:],
                            op=mybir.AluOpType.is_equal,
                        )
                        s_tile = selp.tile([P, P], DT_AGG, tag="s")
                        nc.vector.tensor_scalar_mul(
                            s_tile[:], sel[:], ecoef_sb[:, ch : ch + 1]
                        )
                        for half in range(2):
                            nc.tensor.matmul(
                                psum[:, half * 512 : (half + 1) * 512],
                                lhsT=s_tile[:],
                                rhs=g_tile[:, half * 512 : (half + 1) * 512],
                                start=(i == 0),
                                stop=(i == n - 1),
                            )
                        ch += 1
                    stage = asp.tile([P, F], DT_PART, tag="aggst")
                    if n == 0:
                        nc.vector.memset(stage[:], 0)
                    else:
                        nc.vector.tensor_copy(stage[:], psum[:])
                    nc.sync.dma_start(partial[br][w * P : (w + 1) * P, :], stage[:])
                nc.gpsimd.collective_compute(
                    "ReduceScatter",
                    mybir.AluOpType.add,
                    replica_groups=RG,
                    ins=[partial[br][:]],
                    outs=[recv[br][:]],
                )

        # ================= phase D: bias + lrelu + transpose ==============
        GT0 = (DVP + 1024) // P  # first gcn tile row-block in featT (11)
        with (
            tc.tile_pool(name="biasb", bufs=1) as bp,
            tc.tile_pool(name="gfeat", bufs=3) as gf,
            tc.tile_pool(name="gps", bufs=4, space="PSUM") as gpp,
        ):
            # dv/pe blocks into featT via SBUF bounce
            for t in range(DVP // P):
                bt = gf.tile([P, BS], f32, tag="bounce")
                nc.sync.dma_start(bt[:], dv_t[t * P : (t + 1) * P, :])
                nc.sync.dma_start(featT[t * P : (t + 1) * P, :], bt[:])
            for t in range(1024 // P):
                bt = gf.tile([P, BS], f32, tag="bounce")
                nc.sync.dma_start(bt[:], pe_t[t * P : (t + 1) * P, :])
                nc.sync.dma_start(featT[(DVP // P + t) * P : (DVP // P + t + 1) * P, :], bt[:])
            # broadcast gcn bias row across partitions via ones-matmul
            bias_row = bp.tile([1, GCNF], f32)
            nc.sync.dma_start(bias_row[:], gcn_bias[:])
            bias_bc = bp.tile([P, GCNF], f32)
            for ns in range(GCNF // 512):
                pb = gpp.tile([P, 512], f32, tag="pb")
                nc.tensor.matmul(
                    pb[:],
                    lhsT=ones_sb[:],
                    rhs=bias_row[:, ns * 512 : (ns + 1) * 512],
                    start=True,
                    stop=True,
                )
                nc.vector.tensor_copy(bias_bc[:, ns * 512 : (ns + 1) * 512], pb[:])
            for rt in range(BS // P):
                t_acc = gf.tile([P, GCNF], f32, tag="tacc")
                for br in range(NBR):
                    rtile = gf.tile([P, F], DT_PART, tag="rtile")
                    nc.sync.dma_start(rtile[:], recv[br][rt * P : (rt + 1) * P, :])
                    nc.vector.tensor_copy(t_acc[:, br * F : (br + 1) * F], rtile[:])
                nc.vector.tensor_tensor(
                    out=t_acc[:], in0=t_acc[:], in1=bias_bc[:], op=mybir.AluOpType.add
                )
                nc.scalar.activation(t_acc[:], t_acc[:], AF.Lrelu, alpha=0.01)
                nc.sync.dma_start(gcn_feat[rt * P : (rt + 1) * P, :], t_acc[:])
                for ft in range(GCNF // P):
                    tp = gpp.tile([P, P], f32, tag="tp")
                    nc.tensor.transpose(
                        out=tp[:], in_=t_acc[:, ft * P : (ft + 1) * P], identity=identity[:]
                    )
                    ts_ = gf.tile([P, P], f32, tag="tstage")
                    nc.vector.tensor_copy(ts_[:], tp[:])
                    nc.sync.dma_start(
                        featT[(GT0 + ft) * P : (GT0 + ft + 1) * P, rt * P : (rt + 1) * P],
                        ts_[:],
                    )

        # ================= phase E: MLP =================
        OG = 8  # output tiles per group (8 psum banks)

        def dense_bn(name, in_src, in_is_dram, w_dram, din, dout, sidx, act_func,
                     out_sb, out_dram):
            KC, OT = din // P, dout // P
            with (
                tc.tile_pool(name=f"{name}_w", bufs=3) as wp,
                tc.tile_pool(name=f"{name}_rhs", bufs=4) as rp,
                tc.tile_pool(name=f"{name}_z", bufs=OT) as zp,
                tc.tile_pool(name=f"{name}_sc", bufs=2) as scp,
                tc.tile_pool(name=f"{name}_st", bufs=1) as stp,
                tc.tile_pool(name=f"{name}_ps", bufs=1, space="PSUM") as pp,
            ):
                sums = stp.tile([P, OT], f32)
                sumsq = stp.tile([P, OT], f32)
                zs = []
                for og0 in range(0, OT, OG):
                    og_n = min(OG, OT - og0)
                    psums = [pp.tile([P, BS], f32, tag=f"ps{j}", name=f"{name}_ps{j}") for j in range(og_n)]
                    for kc in range(KC):
                        w_sb = wp.tile([P, OG * P], f32, tag="w")
                        nc.sync.dma_start(
                            w_sb[:, : og_n * P],
                            w_dram[kc * P : (kc + 1) * P, og0 * P : (og0 + og_n) * P],
                        )
                        if in_is_dram:
                            rhs = rp.tile([P, BS], f32, tag="rhs")
                            nc.sync.dma_start(rhs[:], in_src[kc * P : (kc + 1) * P, :])
                            rhs_ap = rhs[:]
                        else:
                            rhs_ap = in_src[:, kc, :]
                        for j in range(og_n):
                            nc.tensor.matmul(
                                psums[j][:],
                                lhsT=w_sb[:, j * P : (j + 1) * P],
                                rhs=rhs_ap,
                                start=(kc == 0),
                                stop=(kc == KC - 1),
                            )
                    for j in range(og_n):
                        ot = og0 + j
                        z = zp.tile([P, BS], f32, tag="z")
                        nc.scalar.activation(
                            z[:], psums[j][:], AF.Identity, accum_out=sums[:, ot : ot + 1]
                        )
                        scr = scp.tile([P, BS], f32, tag="scr")
                        nc.scalar.activation(
                            scr[:], psums[j][:], AF.Square, accum_out=sumsq[:, ot : ot + 1]
                        )
                        zs.append(z)
                nc.sync.dma_start(stats_in[sidx][0:P, :], sums[:])
                nc.sync.dma_start(stats_in[sidx][P : 2 * P, :], sumsq[:])
                nc.gpsimd.collective_compute(
                    "AllReduce",
                    mybir.AluOpType.add,
                    replica_groups=RG,
                    ins=[stats_in[sidx][:]],
                    outs=[stats_out[sidx][:]],
                )
                asum = stp.tile([P, OT], f32)
                asq = stp.tile([P, OT], f32)
                nc.sync.dma_start(asum[:], stats_out[sidx][0:P, :])
                nc.sync.dma_start(asq[:], stats_out[sidx][P : 2 * P, :])
                g_sb = stp.tile([P, OT], f32)
                b_sb = stp.tile([P, OT], f32)
                nc.sync.dma_start(g_sb[:], bn_g[sidx][:])
                nc.sync.dma_start(b_sb[:], bn_b[sidx][:])
                m = stp.tile([P, OT], f32)
                nc.vector.tensor_scalar_mul(m[:], asum[:], 1.0 / B)
                ex2 = stp.tile([P, OT], f32)
                nc.vector.tensor_scalar_mul(ex2[:], asq[:], 1.0 / B)
                m2 = stp.tile([P, OT], f32)
                nc.vector.tensor_tensor(
                    out=m2[:], in0=m[:], in1=m[:], op=mybir.AluOpType.mult
                )
                v = stp.tile([P, OT], f32)
                nc.vector.tensor_tensor(
                    out=v[:], in0=ex2[:], in1=m2[:], op=mybir.AluOpType.subtract
                )
                sd = stp.tile([P, OT], f32)
                nc.scalar.activation(sd[:], v[:], AF.Sqrt, bias=eps_sb[:])
                inv = stp.tile([P, OT], f32)
                nc.vector.reciprocal(inv[:], sd[:])
                sc = stp.tile([P, OT], f32)
                nc.vector.tensor_tensor(
                    out=sc[:], in0=g_sb[:], in1=inv[:], op=mybir.AluOpType.mult
                )
                msc = stp.tile([P, OT], f32)
                nc.vector.tensor_tensor(
                    out=msc[:], in0=m[:], in1=sc[:], op=mybir.AluOpType.mult
                )
                tt = stp.tile([P, OT], f32)
                nc.vector.tensor_tensor(
                    out=tt[:], in0=b_sb[:], in1=msc[:], op=mybir.AluOpType.subtract
                )
                for ot in range(OT):
                    dst = out_sb[:, ot, :] if out_sb is not None else zs[ot][:]
                    nc.scalar.activation(
                        dst,
                        zs[ot][:],
                        act_func,
                        scale=sc[:, ot : ot + 1],
                        bias=tt[:, ot : ot + 1],
                        alpha=0.01,
                    )
                    if out_dram is not None:
                        nc.sync.dma_start(out_dram[ot * P : (ot + 1) * P, :], dst)

        with tc.tile_pool(name="mlp_acts", bufs=1) as ap_:
            enc_sb = ap_.tile([P, 1024 // P, BS], f32)
            h5 = ap_.tile([P, 256 // P, BS], f32)
            with tc.tile_pool(name="mlp_h1", bufs=1) as m1:
                h1 = m1.tile([P, 2048 // P, BS], f32)
                dense_bn("enc1", featT, True, w1, DIMP, 2048, 1, AF.Relu, h1, None)
                dense_bn("enc2", h1, False, w2, 2048, 1024, 2, AF.Relu, enc_sb, enc_t)
            with tc.tile_pool(name="mlp_h3", bufs=1) as m3:
                h3 = m3.tile([P, 2048 // P, BS], f32)
                dense_bn("dec1", enc_sb, False, w3, 1024, 2048, 3, AF.Relu, h3, None)
                dense_bn("dec2", h3, False, w4, 2048, DIMP, 4, AF.Relu, None, dec_t)
            dense_bn("out1", enc_sb, False, w5, 1024, 256, 5, AF.Lrelu, h5, None)

            with (
                tc.tile_pool(name="out2", bufs=1) as op_,
                tc.tile_pool(name="out2ps", bufs=1, space="PSUM") as opp,
            ):
                w6_sb = op_.tile([P, 2, 1], f32)
                for c_ in range(2):
                    nc.sync.dma_start(w6_sb[:, c_, :], w6[c_ * P : (c_ + 1) * P, :])
                yp = opp.tile([1, BS], f32)
                for c_ in range(2):
                    nc.tensor.matmul(
                        yp[:],
                        lhsT=w6_sb[:, c_, :],
                        rhs=h5[:, c_, :],
                        start=(c_ == 0),
                        stop=(c_ == 1),
                    )
                y_sb = op_.tile([1, BS], f32)
                nc.vector.tensor_scalar_add(y_sb[:], yp[:], float(meta["b6"]))
                nc.sync.dma_start(y_t[:], y_sb[:])

    nc.compile()
    return nc


def _prog_key(meta):
    return (
        tuple(tuple(w) for w in meta["win_chunks"]),
        meta["b6"],
        AGG_BF16,
        PART_BF16,
        MLP_BF16,
    )


def _get_program(meta):
    key = _prog_key(meta)
    if key not in _PROG_CACHE:
        _PROG_CACHE.clear()
        _PROG_CACHE[key] = _build_program(meta)
    return _PROG_CACHE[key]


def _run_on_hw(per_core, meta, trace):
    _install_ntff_hook()
    from concourse.bass_utils import run_bass_kernel_spmd

    nc = _get_program(meta)
    in_maps = [
        {k: np.ascontiguousarray(v) for k, v in ci.items()} for ci in per_core
    ]
    res = run_bass_kernel_spmd(
        nc, in_maps, core_ids=list(range(N_CORES)), trace=trace
    )
    results = [
        dict(
            gcn_feat=r["gcn_feat"].astype(np.float32),
            enc_t=r["enc_t"].astype(np.float32),
            dec_t=r["dec_t"].astype(np.float32),
            y_t=r["y_t"].astype(np.float32),
        )
        for r in res.results
    ]
    return results, res.exec_time_ns
